# revision 53
# baseline (speedup 1.0000x reference)
"""Trainium2 Bass kernel for MixedCausalAttention (16 heads, d=1024, L_S=4096, L_NS=64).

Sharding: tensor-parallel over heads - 2 heads per core x 8 cores.
Each core computes qkv projections (shared W_S for S tokens, per-token W_NS for
NS tokens) for its 2 heads, causal attention, and a partial W_out product over
its 128 output feature rows. The host sums the 8 partial (2112, 1024) outputs.

Engine budget strategy (per core, timeline-sim calibrated):
- PE: S-token projections stream x^T (bf16) against W_S stationaries; V is
  produced in natural [s, dh] layout directly (x^T chunk as stationary) so no
  PE transposes are needed. The 805MB W_NS stream is fp8 (scaled 32x to dodge
  e4m3 subnormals) and consumed by DoubleRow matmuls (2 contraction chunks per
  pass at 0.5 cycles/row). A zero-padded selector stationary (xmask) places
  each NS token's projection in its own PSUM partition so 32 tokens accumulate
  into one bank, replacing 64 single-row staging copies with 2 wide ones.
- Act: exp over [128, 2heads, qw] two-bank PSUM tiles (one instruction per
  k-chunk, both heads) - ~96us, the engine's only job (plus a few tail-end
  PSUM->SBUF copies where it is otherwise idle).
- DVE: PSUM->SBUF staging (bf16 outputs), causal masking via precomputed
  staircase mask multiply (only the 128 window columns that cross the
  diagonal), softmax normalization mul.
- Softmax denominators come from an all-ones 65th column in the V stationary;
  the reciprocal row is broadcast across partitions with a K=1 f32r PE matmul
  (NB: the gpsimd partition_broadcast ISA op reads physical partition 0
  regardless of the AP on real HW - do not use it for this).
- Diagonal q-chunks are windowed: columns below the staircase are skipped in
  scores/exp/AV entirely.
- PSUM pitfall baked in everywhere: matmul start=True marks the WHOLE 2KB
  bank pending-zero, so only the first write to a bank may set it; later
  disjoint sub-writes accumulate onto pending-zero bytes.
- The NS-query q-tile (64 queries) runs interleaved with the last S q-tile,
  its two heads' AV packed into one PSUM bank at different free offsets.

Dtypes: x/W_S/K/Q/V/exp/W_out in bf16 (~0.4% rel err paths), W_NS+x_NS in
fp8-e4m3 (affects only the 64 NS query rows, ~3% there, ~0.5% overall),
output bf16 partials summed on host in fp32. Measured end-to-end rel err vs
the fp32 reference: ~6.6e-3 (budget 2e-2).
"""

import os
import sys
import math
import contextlib
from concurrent.futures import ThreadPoolExecutor

for _p in ("/opt/trn_rl_repo", "/root/.axon_site/_ro/trn_rl_repo"):
    if os.path.isdir(_p) and _p not in sys.path:
        sys.path.insert(0, _p)

import numpy as np
import ml_dtypes

import concourse.bass as bass
import concourse.mybir as mybir
import concourse.tile as tile
from concourse import bacc
from concourse.bass_utils import run_bass_kernel_spmd
from concourse.masks import make_identity

F32 = mybir.dt.float32
F32R = mybir.dt.float32r
BF16 = mybir.dt.bfloat16
F8 = mybir.dt.float8e4

N_CORES = 8
D = 1024
H = 16
DH = 64
HPC = H // N_CORES          # heads per core = 2
O3 = 3 * DH * HPC           # 384 qkv output cols per core
LNS = 64
LS = 4096
QS = 2048                   # query_start
LQ = LS - QS + LNS          # 2112 queries
NCH = D // 128              # 8 contraction chunks
ST = 512                    # s-tile width for projections
QT = 512                    # q-tile width for attention
SCALE = DH ** -0.5
NKC = LS // 128             # 32 S key chunks
GT = 32                     # NS-token group size (PSUM partition batch)
WNS_SCALE = 32.0            # fp8 pre-scale for W_NS (dodges e4m3 subnormals)
lqs = LS - QS               # 2048 S-query columns


def build_program(repeat=1):
    nc = bacc.Bacc("TRN2", target_bir_lowering=False, debug=False,
                   num_devices=N_CORES)

    xt_d = nc.dram_tensor("xt", [128, NCH, LS], BF16, kind="ExternalInput")
    xmask_d = nc.dram_tensor("xmask", [128, 4, 2, LNS, GT], F8,
                             kind="ExternalInput")
    ws_d = nc.dram_tensor("ws", [128, NCH, O3], BF16, kind="ExternalInput")
    wns_d = nc.dram_tensor("wns", [LNS, 128, NCH, O3], F8,
                           kind="ExternalInput")
    wout_d = nc.dram_tensor("wout", [128, D], BF16, kind="ExternalInput")
    vones_d = nc.dram_tensor("vones", [65, 64], F32R, kind="ExternalInput")
    o_d = nc.dram_tensor("o", [LQ, D], BF16, kind="ExternalOutput")

    n_kc = NKC + 1           # + NS chunk
    DR = mybir.MatmulPerfMode.DoubleRow

    with tile.TileContext(nc) as tc:
      for _rep in range(repeat):
        with contextlib.ExitStack() as ctx:
            const = ctx.enter_context(tc.tile_pool(name="const", bufs=1))
            store = ctx.enter_context(tc.tile_pool(name="store", bufs=1))

            # --- constants (ws first: the proj matmuls need it + xt0 only;
            # xmask/wout stream in behind the first x tiles) ---
            ws_sb = const.tile([128, NCH, O3], BF16)
            nc.sync.dma_start(out=ws_sb, in_=ws_d.ap())
            wout_sb = const.tile([128, D], BF16)
            ident_sb = const.tile([64, 64], F32)
            make_identity(nc, ident_sb[:, :])

            # causal staircase masks, one per diagonal offset d: keep iff
            # -128*d - k_row + q_col >= 0, replicated over the 2-head slot dim
            ones_sb = const.tile([128, 2, 512], BF16)
            nc.vector.memset(ones_sb[:, :, :], 1.0)
            masks_sb = const.tile([128, 4, 2, 512], BF16)
            for d in range(4):
                nc.gpsimd.affine_select(
                    out=masks_sb[:, d], in_=ones_sb[:, :, :],
                    compare_op=mybir.AluOpType.is_ge, fill=0.0,
                    base=-128 * d, channel_multiplier=-1,
                    pattern=[[0, 2], [1, 512]])
            # prewarm the Act exp table so the 1.3us load is off the
            # critical path
            warm_sb = const.tile([1, 4], F32)
            nc.scalar.activation(
                out=warm_sb[0:1, 0:1], in_=ones_sb[0:1, 0, 0:1],
                func=mybir.ActivationFunctionType.Exp, scale=SCALE)
            # f32r all-ones row at partition 64: stationary for the PE
            # reciprocal-row broadcast (rc lives at partition 64, and matmul
            # requires lhsT/rhs partition bases to match). memset can't emit
            # f32r, so DMA it from a tiny DRAM constant.
            ones64_sb = const.tile([65, 64], F32R)
            nc.sync.dma_start(out=ones64_sb, in_=vones_d.ap())

            # --- persistent activation storage ---
            qt_s = store.tile([128, lqs], BF16)     # Q^T S part (h0 0-63, h1 64-127)
            qt_ns = store.tile([128, LNS], BF16)
            kt_s = store.tile([128, LS], BF16)
            kt_ns = store.tile([128, LNS], BF16)
            v_s = [store.tile([128, NKC, 65], BF16, name=f"v_s{h}")
                   for h in range(2)]
            v_ns = [store.tile([64, 65], BF16, name=f"v_ns{h}") for h in range(2)]
            qkvns_sb = store.tile([64, O3], F32)    # natural-layout NS qkv rows

            # ones columns for the denominator trick
            for h in range(2):
                nc.vector.memset(v_s[h][:, :, 64:65], 1.0)
                nc.vector.memset(v_ns[h][:, 64:65], 1.0)

            # ---------------- NS-token projections (emitted interleaved) ----
            wnspool = ctx.enter_context(tc.tile_pool(name="wnspool", bufs=10))
            psNS = ctx.enter_context(tc.tile_pool(name="psNS", bufs=1,
                                                  space="PSUM"))

            xmask_holder = []

            def ns_emitter():
                xmask_sb = xmask_holder[0]
                for g in range(LNS // GT):
                    psn = psNS.tile([128, 512], F32, tag="psNS")
                    for j in range(GT):
                        n = GT * g + j
                        wns_t = wnspool.tile([128, NCH, O3], F8, tag="wns")
                        nc.sync.dma_start(out=wns_t, in_=wns_d.ap()[n])
                        for cp in range(NCH // 2):
                            nc.tensor.matmul(
                                psn[0:GT, 0:O3],
                                lhsT=xmask_sb[:, cp, :, n, :],
                                rhs=wns_t[:, 2 * cp:2 * cp + 2, :],
                                start=(j == 0 and cp == 0),
                                stop=(j == GT - 1 and cp == NCH // 2 - 1),
                                perf_mode=DR)
                        yield
                    nc.vector.tensor_scalar_mul(
                        qkvns_sb[GT * g:GT * (g + 1), :], psn[0:GT, 0:O3],
                        1.0 / WNS_SCALE)
                # finalize: Q_NS^T / K_NS^T via PE transpose, V_NS natural
                for part, dest in ((0, qt_ns), (1, kt_ns)):
                    pst = psNS.tile([128, 512], F32, tag="psNS")
                    nc.tensor.transpose(
                        pst[0:128, 0:64],
                        qkvns_sb[0:64, part * 128:(part + 1) * 128],
                        ident_sb[:, :])
                    nc.vector.tensor_copy(out=dest[:, :], in_=pst[0:128, 0:64])
                for h in range(2):
                    nc.vector.tensor_copy(
                        out=v_ns[h][0:64, 0:64],
                        in_=qkvns_sb[0:64, 256 + h * 64:256 + (h + 1) * 64])
                while True:
                    yield

            ns_gen = ns_emitter()
            ns_left = LNS + 1  # token steps + finalize step

            # ---------------- stage A: S-token projections ----------------
            xpool = ctx.enter_context(tc.tile_pool(name="xpool", bufs=2))

            def load_xt(st):
                # high_priority: the scheduler must not queue wns/xmask
                # prefetches ahead of the x tiles the projections block on
                t = xpool.tile([128, NCH, ST], BF16, tag="xt")
                with tc.high_priority():
                    if st == 0:
                        # halve the critical first-matmul DMA wait
                        nc.sync.dma_start(out=t[:, 0:4],
                                          in_=xt_d.ap()[:, 0:4, 0:ST])
                        nc.sync.dma_start(out=t[:, 4:8],
                                          in_=xt_d.ap()[:, 4:8, 0:ST])
                    else:
                        nc.sync.dma_start(
                            out=t, in_=xt_d.ap()[:, :, st * ST:(st + 1) * ST])
                return t

            xt_next = load_xt(0)
            with tc.tile_pool(name="psA", bufs=2, space="PSUM") as psA:
                for st in range(LS // ST):
                    s0 = st * ST
                    xt_t = xt_next
                    if st + 1 < LS // ST:
                        xt_next = load_xt(st + 1)
                    if st == 3:
                        # deferred constants; needed from the attention phase.
                        # xmask tile created here so the scheduler doesn't
                        # hoist its DMAs ahead of the x-tile reloads
                        xmask_sb = const.tile([128, 4, 2, LNS, GT], F8)
                        xmask_holder.append(xmask_sb)
                        for cp in range(4):
                            nc.sync.dma_start(out=xmask_sb[:, cp],
                                              in_=xmask_d.ap()[:, cp])
                        nc.sync.dma_start(out=wout_sb, in_=wout_d.ap())
                    jobs = [(1, kt_s, s0)]
                    if s0 >= QS:
                        jobs.append((0, qt_s, s0 - QS))
                    for mi, dest, dcol in jobs:
                        ps = psA.tile([128, ST], F32, tag="psA")
                        for ci in range(NCH):
                            nc.tensor.matmul(
                                ps[:, :],
                                lhsT=ws_sb[:, ci, mi * 128:(mi + 1) * 128],
                                rhs=xt_t[:, ci, :],
                                start=(ci == 0), stop=(ci == NCH - 1))
                        nc.vector.tensor_copy(out=dest[:, dcol:dcol + ST],
                                              in_=ps[:, :])
                    # V in natural [s, dh] layout: x^T chunk as stationary
                    # NB: start=True marks the whole 2KB PSUM bank pending-zero,
                    # so only the first write in the bank may set it; later
                    # sub-regions accumulate onto pending-zero (reads as 0).
                    psv = psA.tile([128, 4, 128], F32, tag="psV")
                    for sub in range(ST // 128):
                        for ci in range(NCH):
                            nc.tensor.matmul(
                                psv[:, sub, :],
                                lhsT=xt_t[:, ci, sub * 128:(sub + 1) * 128],
                                rhs=ws_sb[:, ci, 256:384],
                                start=(sub == 0 and ci == 0),
                                stop=(sub == ST // 128 - 1 and ci == NCH - 1),
                                skip_group_check=True)
                    for h in range(2):
                        nc.vector.tensor_copy(
                            out=v_s[h][:, st * 4:(st + 1) * 4, 0:64],
                            in_=psv[:, :, h * 64:(h + 1) * 64])

            # ---------------- main attention loop ----------------
            expool = ctx.enter_context(tc.tile_pool(name="expool", bufs=4))
            nrm = ctx.enter_context(tc.tile_pool(name="nrm", bufs=2))
            avpool = ctx.enter_context(tc.tile_pool(name="avpool", bufs=2))
            outpool = ctx.enter_context(tc.tile_pool(name="outpool", bufs=4))
            psS = ctx.enter_context(tc.tile_pool(name="psS", bufs=2, space="PSUM"))
            psAV = ctx.enter_context(tc.tile_pool(name="psAV", bufs=2, space="PSUM"))
            psO = ctx.enter_context(tc.tile_pool(name="psO", bufs=1, space="PSUM"))

            def emit_norm_wout(get_av, get_den, q0, qw, alt_pool=None):
                """softmax normalization (reciprocal of the ones-column sums,
                PE row-broadcast, multiply) + W_out partial + output DMA.
                alt_pool doubles the PSUM staging ring for the final q-tile."""
                def po_tile(i):
                    if alt_pool is not None and i % 2 == 1:
                        return alt_pool.tile([128, 512], F32, tag="psNS",
                                             name="po_alt")
                    return psO.tile([128, 512], F32, tag="po", name="po")
                av_cat = avpool.tile([128, 512], BF16, tag="av")
                for h in range(2):
                    rc = nrm.tile([65, 512], F32R, tag="rc")
                    with nc.allow_low_precision(reason="f32r recip row feeds "
                                                "full-rate PE broadcast"):
                        nc.vector.reciprocal(out=rc[64:65, 0:qw],
                                             in_=get_den(h))
                    pbc = po_tile(h)
                    nc.tensor.matmul(pbc[0:64, 0:qw],
                                     lhsT=ones64_sb[64:65, 0:64],
                                     rhs=rc[64:65, 0:qw],
                                     start=True, stop=True)
                    bc = nrm.tile([64, 512], F32, tag="bc")
                    if alt_pool is not None:
                        nc.scalar.copy(out=bc[0:64, 0:qw], in_=pbc[0:64, 0:qw])
                    else:
                        nc.vector.tensor_copy(out=bc[0:64, 0:qw],
                                              in_=pbc[0:64, 0:qw])
                    nc.vector.tensor_mul(av_cat[h * 64:(h + 1) * 64, 0:qw],
                                         get_av(h), bc[0:64, 0:qw])
                for qs in range(math.ceil(qw / 128)):
                    qsw = min(128, qw - qs * 128)
                    ot = outpool.tile([128, D], BF16, tag="ot")
                    for e in range(2):
                        po = po_tile(qs * 2 + e)
                        nc.tensor.matmul(
                            po[0:qsw, :],
                            lhsT=av_cat[:, qs * 128:qs * 128 + qsw],
                            rhs=wout_sb[:, e * 512:(e + 1) * 512],
                            start=True, stop=True)
                        if alt_pool is not None and e == 1:
                            nc.scalar.copy(
                                out=ot[0:qsw, e * 512:(e + 1) * 512],
                                in_=po[0:qsw, :])
                        else:
                            nc.vector.tensor_copy(
                                out=ot[0:qsw, e * 512:(e + 1) * 512],
                                in_=po[0:qsw, :])
                    nc.sync.dma_start(
                        out=o_d.ap()[q0 + qs * 128:q0 + qs * 128 + qsw, :],
                        in_=ot[0:qsw, :])

            def ns_attn_emitter():
                # NS queries (64): both heads' AV accumulate in ONE psNS bank
                # at different free offsets; 8 k-chunks share each exp
                qw4 = LNS
                ps_av4 = psNS.tile([128, 512], F32, tag="psNS")
                n_groups = math.ceil(n_kc / 8)
                for cg in range(n_groups):
                    chunks = list(range(8 * cg, min(8 * cg + 8, n_kc)))
                    ps_s = psS.tile([128, 2, 512], F32, tag="psS")
                    gw = 64 * len(chunks)
                    for i, kc in enumerate(chunks):
                        kw = LNS if kc == NKC else 128
                        for h in range(2):
                            hs = slice(h * 64, (h + 1) * 64)
                            if kc == NKC:
                                lh = kt_ns[hs, 0:kw]
                            else:
                                lh = kt_s[hs, kc * 128:(kc + 1) * 128]
                            nc.tensor.matmul(
                                ps_s[0:kw, h, 64 * i:64 * i + qw4],
                                lhsT=lh, rhs=qt_ns[hs, 0:qw4],
                                start=(i == 0), stop=(i == len(chunks) - 1),
                                skip_group_check=True)
                    gkw = LNS if chunks == [NKC] else 128
                    ex = expool.tile([128, 2, 512], BF16, tag="ex")
                    nc.scalar.activation(
                        out=ex[0:gkw, :, 0:gw], in_=ps_s[0:gkw, :, 0:gw],
                        func=mybir.ActivationFunctionType.Exp, scale=SCALE)
                    for i, kc in enumerate(chunks):
                        kw = LNS if kc == NKC else 128
                        if kc == NKC:
                            # NS-NS corner: keep iff q >= k row
                            nc.vector.tensor_tensor(
                                out=ex[0:kw, :, 64 * i:64 * i + qw4],
                                in0=ex[0:kw, :, 64 * i:64 * i + qw4],
                                in1=masks_sb[0:kw, 0, :, 0:qw4],
                                op=mybir.AluOpType.mult)
                        for h in range(2):
                            v_src = v_ns[h][0:kw, 0:65] if kc == NKC \
                                else v_s[h][0:kw, kc, 0:65]
                            nc.tensor.matmul(
                                ps_av4[0:65, 64 * h:64 * h + 64],
                                lhsT=v_src,
                                rhs=ex[0:kw, h, 64 * i:64 * i + qw4],
                                start=(cg == 0 and i == 0 and h == 0),
                                stop=(kc == NKC and h == 1),
                                skip_group_check=True)
                    yield
                emit_norm_wout(
                    lambda h: ps_av4[0:64, 64 * h:64 * h + 64],
                    lambda h: ps_av4[64:65, 64 * h:64 * h + 64],
                    lqs, qw4)
                while True:
                    yield

            q_tiles = [(q0, QT) for q0 in range(0, lqs, QT)]
            ns_attn_gen = None
            ns_attn_left = 0
            for qt_i, (q0, qw) in enumerate(q_tiles):
                kc_count = min((QS + q0 + qw - 1) // 128 + 1, n_kc)
                ps_av = [psAV.tile([65, 512], F32, tag="psAV", name=f"av{h}")
                         for h in range(2)]
                base_kc = (QS + q0) // 128

                if qt_i == 3:
                    # tokens finished during earlier q-tiles; finalize and arm
                    # the NS-query attention to interleave with this q-tile
                    while ns_left > 0:
                        next(ns_gen)
                        ns_left -= 1
                    ns_attn_gen = ns_attn_emitter()
                    ns_attn_left = math.ceil(n_kc / 8) + 1

                if True:
                    for kc in range(kc_count):
                        # diagonal chunk at offset d: columns q < 128d are
                        # fully masked - skip them in scores/exp/mask/AV
                        d = kc - base_kc
                        qlo = 128 * d if d > 0 else 0
                        ps_s = psS.tile([128, 2, 512], F32, tag="psS")
                        for h in range(2):
                            hs = slice(h * 64, (h + 1) * 64)
                            nc.tensor.matmul(
                                ps_s[:, h, qlo:qw],
                                lhsT=kt_s[hs, kc * 128:(kc + 1) * 128],
                                rhs=qt_s[hs, q0 + qlo:q0 + qw],
                                start=True, stop=True)
                        ex = expool.tile([128, 2, 512], BF16, tag="ex")
                        nc.scalar.activation(
                            out=ex[:, :, qlo:qw], in_=ps_s[:, :, qlo:qw],
                            func=mybir.ActivationFunctionType.Exp, scale=SCALE)
                        if d >= 0:
                            # only the first 128 window columns cross the
                            # staircase; the rest are fully kept
                            nc.vector.tensor_tensor(
                                out=ex[:, :, qlo:qlo + 128],
                                in0=ex[:, :, qlo:qlo + 128],
                                in1=masks_sb[:, d, :, qlo:qlo + 128],
                                op=mybir.AluOpType.mult)
                        for h in range(2):
                            nc.tensor.matmul(
                                ps_av[h][0:65, qlo:qw],
                                lhsT=v_s[h][:, kc, :],
                                rhs=ex[:, h, qlo:qw],
                                start=(kc == 0), stop=(kc == kc_count - 1),
                                skip_group_check=True)
                        if qt_i < 3:
                            if ns_left > 1:
                                next(ns_gen)
                                ns_left -= 1
                        elif ns_attn_left > 0 and kc % 6 == 5:
                            next(ns_attn_gen)
                            ns_attn_left -= 1

                if qt_i == 3:
                    # finish NS-query attention (incl its norm/W_out) so the
                    # psNS bank is free to double this q-tile's staging ring
                    while ns_attn_left > 0:
                        next(ns_attn_gen)
                        ns_attn_left -= 1
                emit_norm_wout(lambda h: ps_av[h][0:64, 0:qw],
                               lambda h: ps_av[h][64:65, 0:qw], q0, qw,
                               alt_pool=psNS if qt_i == 3 else None)

    nc.compile()
    return nc


_NC_CACHE = {}


def _get_program():
    if "nc" not in _NC_CACHE:
        _NC_CACHE["nc"] = build_program()
    return _NC_CACHE["nc"]


def prep_shared(x):
    """Shared (core-independent) input tensors."""
    xs = x[0]
    x2 = xs[:LS].T.reshape(NCH, 128, LS).transpose(1, 0, 2)
    xt = np.ascontiguousarray(x2.astype(ml_dtypes.bfloat16))  # (128, 8, 4096)

    # xmask[p, cp, j, n, t] = x_NS[n, (2cp+j)*128+p] if t == n % GT else 0
    xns = xs[LS:]                                      # (64, 1024)
    xm = np.zeros((128, 4, 2, LNS, GT), dtype=ml_dtypes.float8_e4m3)
    xc = xns.T.reshape(4, 2, 128, LNS).transpose(2, 0, 1, 3)  # (128,4,2,64)
    xc8 = xc.astype(ml_dtypes.float8_e4m3)
    for n in range(LNS):
        xm[:, :, :, n, n % GT] = xc8[:, :, :, n]
    return xt, xm


def _prep_core(c, xt, xm, W_S, W_NS, W_out):
    """Host-side shard prep for core c (heads 2c, 2c+1)."""
    h0 = 2 * c * DH
    cols = np.r_[h0:h0 + HPC * DH,
                 D + h0:D + h0 + HPC * DH,
                 2 * D + h0:2 * D + h0 + HPC * DH]
    ws = W_S[:, cols].reshape(NCH, 128, O3).transpose(1, 0, 2)
    ws = np.ascontiguousarray(ws.astype(ml_dtypes.bfloat16))
    wns = W_NS[:, :, cols].reshape(LNS, NCH, 128, O3).transpose(0, 2, 1, 3)
    wns = np.ascontiguousarray(
        (wns * WNS_SCALE).astype(ml_dtypes.float8_e4m3))
    wout = np.ascontiguousarray(
        W_out[h0:h0 + 2 * DH].astype(ml_dtypes.bfloat16))
    return {"xt": xt, "xmask": xm, "ws": ws, "wns": wns, "wout": wout,
            "vones": np.ones((65, 64), dtype=np.float32)}


def prep_in_maps(x, W_S, W_NS, W_out):
    xt, xm = prep_shared(x)
    with ThreadPoolExecutor(max_workers=N_CORES) as ex:
        return list(ex.map(
            lambda c: _prep_core(c, xt, xm, W_S, W_NS, W_out),
            range(N_CORES)))


def kernel(x, W_S, W_NS, W_out, L_S=None, query_start=None, **_unused):
    x = np.asarray(x, dtype=np.float32)
    W_S = np.asarray(W_S, dtype=np.float32)
    W_NS = np.asarray(W_NS, dtype=np.float32)
    W_out = np.asarray(W_out, dtype=np.float32)
    if L_S is not None:
        assert int(L_S) == LS, f"kernel hardcodes L_S={LS}, got {int(L_S)}"
    if query_start is not None:
        assert int(query_start) == QS, \
            f"kernel hardcodes query_start={QS}, got {int(query_start)}"
    assert x.shape == (1, LS + LNS, D)

    nc = _get_program()
    in_maps = prep_in_maps(x, W_S, W_NS, W_out)

    res = None
    for attempt in range(3):
        try:
            res = run_bass_kernel_spmd(nc, in_maps, list(range(N_CORES)))
            break
        except Exception:
            if attempt == 2:
                raise
            # transient device wedges (NRT_EXEC_UNIT_UNRECOVERABLE) have been
            # observed to clear after the terminal resets the session
            import time
            time.sleep(100)
    out = np.zeros((LQ, D), dtype=np.float32)
    for r in res.results:
        out += np.asarray(r["o"], dtype=np.float32)
    return out.reshape(1, LQ, D)


if __name__ == "__main__":
    rng = np.random.default_rng(0)
    ins = {
        "x": rng.standard_normal((1, LS + LNS, D), dtype=np.float32),
        "W_S": rng.standard_normal((D, 3 * D), dtype=np.float32) * 0.02,
        "W_NS": rng.standard_normal((LNS, D, 3 * D), dtype=np.float32) * 0.02,
        "W_out": rng.standard_normal((D, D), dtype=np.float32) * 0.03,
        "L_S": LS, "query_start": QS,
    }
    out = kernel(**ins)
    print("kernel out shape:", out.shape, "finite:", np.isfinite(out).all())
